# revision 12
# baseline (speedup 1.0000x reference)
"""Paged GQA decode attention, v6: page-granularity gathers (8KB rows).

Per core (kv head): K^T via dma_gather(transpose=True) per 2 seqs
(num_idxs=128 page rows of 32 tokens); V via indirect_dma_start per 2 seqs
([128,4096] 2D dest, one page per partition). Score column t*64+i <->
position 32i+t (t=token-in-page 0..31, i=page 0..63). PV contracts pages
(K=64) with row+col tiled matmuls; pT tiles hold the transpose replicated
in both partition halves so lhsT/rhs partition bases match.
"""

import numpy as np
import ml_dtypes

import concourse.bass as bass
import concourse.bacc as bacc
import concourse.mybir as mybir
from concourse.bass_utils import run_bass_kernel_spmd

B, H, HKV, D = 64, 32, 8, 128
PAGE, PAGES_PER_SEQ, NUM_PAGES = 32, 64, 4096
SMAX = PAGES_PER_SEQ * PAGE  # 2048
NSLOTS = NUM_PAGES * PAGE  # 131072
SCALE = 0.08838834764831843
G = H // HKV
NCORES = 8
GROUPS = 2
GB = B // GROUPS  # 32
NT = PAGE  # 32 token slots per page
NP = PAGES_PER_SEQ  # 64 pages per seq
INVALID_IDX = 1 << 26

NKT = 6  # kT pair-tile ring
NVB = 12  # vbig pair-tile ring

f32 = mybir.dt.float32
bf16 = mybir.dt.bfloat16
i32 = mybir.dt.int32
i16 = mybir.dt.int16
Exp = mybir.ActivationFunctionType.Exp
Copy = mybir.ActivationFunctionType.Copy

# PE counters: 32 qk + 64 ptr (2 per t) + 8 pv u-blocks
PE_PER_GROUP = GB + 2 * NT + 8


def cnt_qk(g2, s):
    return PE_PER_GROUP * g2 + s + 1


def cnt_ptr(g2, k):  # k = 2*t + h
    return PE_PER_GROUP * g2 + GB + k + 1


def cnt_pv(g2, u):
    return PE_PER_GROUP * g2 + GB + 2 * NT + u + 1


NMEMSET = NKT + NVB
DVE_PER_GROUP = 3 + NT


def cnt_add(g2):
    return NMEMSET + DVE_PER_GROUP * g2 + 1


def cnt_recip(g2):
    return cnt_add(g2) + 1


def cnt_pmul(g2):
    return cnt_add(g2) + 2


def cnt_ptcp(g2, t):
    return cnt_add(g2) + 3 + t


ACT_PER_GROUP = 1 + 4


def cnt_exp(g2):
    return ACT_PER_GROUP * g2 + 1


def cnt_ocp(g2, q):
    return ACT_PER_GROUP * g2 + 2 + q


NLOADS = 4 * 16
NIDXLOADS = 3 * 16


def build_nc():
    nc = bacc.Bacc()
    kc = nc.declare_dram_parameter("kc", [NUM_PAGES, PAGE * D], bf16, isOutput=False)
    vc = nc.declare_dram_parameter("vc", [NUM_PAGES, PAGE * D], bf16, isOutput=False)
    qTpad = nc.declare_dram_parameter("qTpad", [D, B * 128], bf16, isOutput=False)
    maskadd = nc.declare_dram_parameter("maskadd", [GROUPS, 128, SMAX], f32, isOutput=False)
    vpidx = nc.declare_dram_parameter("vpidx", [128, B // 2], i32, isOutput=False)
    kidx = nc.declare_dram_parameter("kidx", [128, (B // 2) * 8], i16, isOutput=False)
    kcnt = nc.declare_dram_parameter("kcnt", [1, B // 2], i32, isOutput=False)
    ident_in = nc.declare_dram_parameter("ident", [128, 128], bf16, isOutput=False)
    out = nc.declare_dram_parameter("out", [GROUPS, 16, 8 * D], f32, isOutput=True)

    from contextlib import ExitStack

    with ExitStack() as ctx:
        qT_all = ctx.enter_context(nc.sbuf_tensor("qT_all", [D, B * 128], bf16))
        mask0 = ctx.enter_context(nc.sbuf_tensor("mask0", [128, SMAX], f32))
        mask1 = ctx.enter_context(nc.sbuf_tensor("mask1", [128, SMAX], f32))
        s_t = ctx.enter_context(nc.sbuf_tensor("s_t", [128, SMAX], f32))
        p_t = ctx.enter_context(nc.sbuf_tensor("p_t", [128, SMAX], bf16))
        l_t = ctx.enter_context(nc.sbuf_tensor("l_t", [128, 1], f32))
        rl_t = ctx.enter_context(nc.sbuf_tensor("rl_t", [128, 1], f32))
        idx_all = ctx.enter_context(nc.sbuf_tensor("idx_all", [128, B // 2], i32))
        kidx_sb = ctx.enter_context(nc.sbuf_tensor("kidx_sb", [128, (B // 2) * 8], i16))
        kcnt_sb = ctx.enter_context(nc.sbuf_tensor("kcnt_sb", [1, B // 2], i32))
        identb = ctx.enter_context(nc.sbuf_tensor("identb", [128, 128], bf16))
        kTts = [
            ctx.enter_context(nc.sbuf_tensor(f"kTt{i}", [128, NT, 128], bf16))
            for i in range(NKT)
        ]
        vbig = [
            ctx.enter_context(nc.sbuf_tensor(f"vbig{i}", [128, PAGE * D], bf16))
            for i in range(NVB)
        ]
        pTts = [
            ctx.enter_context(nc.sbuf_tensor(f"pTt{i}", [128, 128], bf16))
            for i in range(NT)
        ]
        ots = [
            ctx.enter_context(nc.sbuf_tensor(f"ot{i}", [128, 8 * D], f32))
            for i in range(GROUPS)
        ]
        scores_ps = ctx.enter_context(nc.psum_tensor("scores_ps", [128, SMAX], f32))
        pvps = ctx.enter_context(nc.psum_tensor("pvps", [128, 8 * D], f32))
        trps = [
            ctx.enter_context(nc.psum_tensor(f"trps{i}", [128, 128], f32))
            for i in range(2)
        ]
        LOADS = ctx.enter_context(nc.semaphore("LOADS"))
        IDXL = ctx.enter_context(nc.semaphore("IDXL"))
        PEs = ctx.enter_context(nc.semaphore("PEs"))
        DVEs = ctx.enter_context(nc.semaphore("DVEs"))
        ACTs = ctx.enter_context(nc.semaphore("ACTs"))
        KS = [ctx.enter_context(nc.semaphore(f"KS{i}")) for i in range(NKT)]
        VS = [ctx.enter_context(nc.semaphore(f"VS{i}")) for i in range(NVB)]
        OS = ctx.enter_context(nc.semaphore("OS"))

        cregs = [nc.alloc_register(mybir.EngineType.Pool, f"creg{i}") for i in range(4)]

        masks = [mask0, mask1]

        with nc.Block() as block:

            @block.sync
            def _(sync):
                sync.dma_start(out=kidx_sb[:, :], in_=kidx[:, :]).then_inc(IDXL, 16)
                sync.dma_start(out=kcnt_sb[:, :], in_=kcnt[:, :]).then_inc(IDXL, 16)
                sync.dma_start(out=idx_all[:, :], in_=vpidx[:, :]).then_inc(IDXL, 16)
                sync.dma_start(out=qT_all[:, :], in_=qTpad[:, :]).then_inc(LOADS, 16)
                sync.dma_start(out=mask0[:, :], in_=maskadd[0]).then_inc(LOADS, 16)
                sync.dma_start(out=mask1[:, :], in_=maskadd[1]).then_inc(LOADS, 16)
                sync.dma_start(out=identb[:, :], in_=ident_in[:, :]).then_inc(LOADS, 16)
                for g2 in range(GROUPS):
                    for q in range(4):
                        sync.wait_ge(ACTs, cnt_ocp(g2, q))
                        sync.dma_start(
                            out=out[g2, 4 * q : 4 * q + 4, :],
                            in_=ots[g2][32 * q : 32 * q + 4, :],
                        ).then_inc(OS, 16)

            @block.gpsimd
            def _(gpsimd):
                bc_reg = gpsimd.to_reg(NUM_PAGES - 1)
                gpsimd.wait_ge(IDXL, NIDXLOADS)
                last_dve = 0
                last_pe = 0

                def wait_dve(v):
                    nonlocal last_dve
                    if v > last_dve:
                        last_dve = v
                        gpsimd.wait_ge(DVEs, v)

                def wait_pe(v):
                    nonlocal last_pe
                    if v > last_pe:
                        last_pe = v
                        gpsimd.wait_ge(PEs, v)

                KOPG = GB // 2  # 16 K ops per group (2 seqs each)
                for g2 in range(GROUPS):
                    for o in range(KOPG):
                        oi = g2 * KOPG + o
                        if oi < NKT:
                            wait_dve(oi + 1)
                        else:
                            ps_ = 2 * (oi - NKT) + 1
                            wait_pe(cnt_qk(ps_ // GB, ps_ % GB))
                        gpsimd.reg_load(cregs[oi % 4], kcnt_sb[0:1, oi : oi + 1])
                        gpsimd.dma_gather(
                            kTts[oi % NKT][:, :, :],
                            kc[:, :],
                            kidx_sb[:, 8 * oi : 8 * oi + 8],
                            2 * NP,
                            cregs[oi % 4],
                            PAGE * D,
                            transpose=True,
                        ).then_inc(KS[oi % NKT], 16)
                    for v in range(KOPG):
                        vi = g2 * KOPG + v
                        if vi < NVB:
                            wait_dve(NKT + vi + 1)
                        else:
                            pv = vi - NVB
                            wait_pe(cnt_pv(pv // KOPG, (pv % KOPG) // 2))
                        gpsimd.indirect_dma_start(
                            out=vbig[vi % NVB][:, :],
                            out_offset=None,
                            in_=vc[:, :],
                            in_offset=bass.IndirectOffsetOnAxis(
                                ap=idx_all[:, vi : vi + 1],
                                axis=0,
                            ),
                            bounds_check=bc_reg,
                            oob_is_err=False,
                        ).then_inc(VS[vi % NVB], 16)

            @block.tensor
            def _(tensor):
                last = {"DVE": 0, "ACT": 0,
                        "KS": [0] * NKT, "VS": [0] * NVB}

                def wait_dve(v):
                    if v > last["DVE"]:
                        last["DVE"] = v
                        tensor.wait_ge(DVEs, v)

                def wait_act(v):
                    if v > last["ACT"]:
                        last["ACT"] = v
                        tensor.wait_ge(ACTs, v)

                def wait_ks(oi):
                    v = 16 * (oi // NKT + 1)
                    if v > last["KS"][oi % NKT]:
                        last["KS"][oi % NKT] = v
                        tensor.wait_ge(KS[oi % NKT], v)

                def wait_vs(vi):
                    v = 16 * (vi // NVB + 1)
                    if v > last["VS"][vi % NVB]:
                        last["VS"][vi % NVB] = v
                        tensor.wait_ge(VS[vi % NVB], v)

                tensor.wait_ge(LOADS, NLOADS)
                for g2 in range(GROUPS):
                    # --- QK: rhs [128, 8, 64] strided slices of pair tile ---
                    for s in range(GB):
                        gi = g2 * GB + s
                        oi, sl = gi // 2, gi % 2
                        wait_ks(oi)
                        if s == 0 and g2 > 0:
                            wait_dve(cnt_add(g2 - 1))
                        lhsT = qT_all[:, gi * 128 : (gi + 1) * 128]
                        for b in range(4):
                            mm = nc.tensor.matmul(
                                out=scores_ps[:, 512 * b : 512 * (b + 1)],
                                lhsT=lhsT,
                                rhs=kTts[oi % NKT][
                                    :, 8 * b : 8 * b + 8, 64 * sl : 64 * sl + 64
                                ],
                                start=(s == 0),
                                stop=(s == GB - 1),
                                skip_group_check=True,
                            )
                        mm.then_inc(PEs, 1)
                    # --- p transposes: per t, both partition halves ---
                    wait_dve(cnt_pmul(g2))
                    for t in range(NT):
                        tj = NT * g2 + t
                        if tj >= 2:
                            pt = tj - 2
                            wait_dve(cnt_ptcp(pt // NT, pt % NT))
                        for h in range(2):
                            nc.tensor.transpose(
                                out=trps[tj % 2][:, :].bitcast(bf16)[
                                    64 * h : 64 * h + 64, :128
                                ],
                                in_=p_t[:, t * 64 : (t + 1) * 64],
                                identity=identb[:, :],
                            ).then_inc(PEs, 1)
                    # --- PV: contraction over 64 pages, row+col tiled ---
                    if g2 > 0:
                        wait_act(cnt_ocp(g2 - 1, 3))
                    for u in range(8):
                        for q_ in range(4):
                            wait_vs(g2 * 16 + (4 * u + q_) // 2)
                        for t in range(NT):
                            wait_dve(cnt_ptcp(g2, t))
                            for q in range(4):
                                s = 4 * u + q
                                sl = s % 2
                                vslot = (g2 * 16 + s // 2) % NVB
                                mm = nc.tensor.matmul(
                                    out=pvps[
                                        32 * q : 32 * q + 4, u * D : (u + 1) * D
                                    ],
                                    lhsT=pTts[t][64 * sl : 64 * sl + 64, 4 * s : 4 * s + 4],
                                    rhs=vbig[vslot][
                                        64 * sl : 64 * sl + 64, t * D : (t + 1) * D
                                    ],
                                    start=(t == 0),
                                    stop=(t == NT - 1),
                                    tile_position=(64 * sl, 32 * q),
                                    skip_group_check=True,
                                )
                        mm.then_inc(PEs, 1)

            @block.vector
            def _(vector):
                # order must match the gpsimd gates: kT i -> DVEs>=i+1,
                # vbig i -> DVEs>=NKT+i+1
                for t_ in kTts:
                    vector.memset(t_[:, :, :], 0.0).then_inc(DVEs, 1)
                for t_ in vbig:
                    vector.memset(t_[:, :], 0.0).then_inc(DVEs, 1)
                last = {"PE": 0, "ACT": 0}

                def wait_pe(v):
                    if v > last["PE"]:
                        last["PE"] = v
                        vector.wait_ge(PEs, v)

                def wait_act(v):
                    if v > last["ACT"]:
                        last["ACT"] = v
                        vector.wait_ge(ACTs, v)

                vector.wait_ge(LOADS, NLOADS)
                for g2 in range(GROUPS):
                    wait_pe(cnt_qk(g2, GB - 1))
                    nc.vector.tensor_add(
                        out=s_t[:, :], in0=scores_ps[:, :], in1=masks[g2][:, :]
                    ).then_inc(DVEs, 1)
                    wait_act(cnt_exp(g2))
                    nc.vector.reciprocal(out=rl_t[:, :], in_=l_t[:, :]).then_inc(DVEs, 1)
                    vector.wait_ge(DVEs, cnt_recip(g2))
                    nc.vector.tensor_scalar_mul(
                        out=p_t[:, :], in0=p_t[:, :], scalar1=rl_t[:, :1]
                    ).then_inc(DVEs, 1)
                    for t in range(NT):
                        wait_pe(cnt_ptr(g2, 2 * t + 1))
                        if g2 > 0 and t == 0:
                            wait_pe(cnt_pv(g2 - 1, 7))
                        nc.vector.tensor_copy(
                            out=pTts[t][:, :],
                            in_=trps[(NT * g2 + t) % 2][:, :].bitcast(bf16)[:, :128],
                        ).then_inc(DVEs, 1)

            @block.scalar
            def _(scalar):
                last = {"PE": 0, "DVE": 0}

                def wait_pe(v):
                    if v > last["PE"]:
                        last["PE"] = v
                        scalar.wait_ge(PEs, v)

                def wait_dve(v):
                    if v > last["DVE"]:
                        last["DVE"] = v
                        scalar.wait_ge(DVEs, v)

                for g2 in range(GROUPS):
                    wait_dve(cnt_add(g2))
                    if g2 > 0:
                        wait_pe(cnt_ptr(g2 - 1, 2 * NT - 1))
                    nc.scalar.activation(
                        out=p_t[:, :], in_=s_t[:, :], func=Exp, accum_out=l_t[:, :1]
                    ).then_inc(ACTs, 1)
                    wait_pe(cnt_pv(g2, 7))
                    for q in range(4):
                        nc.scalar.activation(
                            out=ots[g2][32 * q : 32 * q + 4, :],
                            in_=pvps[32 * q : 32 * q + 4, :],
                            func=Copy,
                        ).then_inc(ACTs, 1)

    nc.compile()
    return nc


_NC_CACHE = None


def _get_nc():
    global _NC_CACHE
    if _NC_CACHE is None:
        _NC_CACHE = build_nc()
    return _NC_CACHE


def make_in_maps(q, k, v, k_cache, v_cache, block_tables, context_lens, slot_mapping):
    q = np.asarray(q, np.float32)
    k = np.asarray(k, np.float32)
    v = np.asarray(v, np.float32)
    k_cache = np.asarray(k_cache, np.float32)
    v_cache = np.asarray(v_cache, np.float32)
    block_tables = np.asarray(block_tables, np.int32)
    context_lens = np.asarray(context_lens, np.int32)
    slot_mapping = np.asarray(slot_mapping, np.int32)

    # slot permutation: per group, even pair-slots (gathered in full)
    # take the longest contexts, odd slots (ucode-truncated) the shortest.
    ord_ = np.zeros(B, np.int64)
    for g2_ in range(GROUPS):
        c_ = context_lens[GB * g2_ : GB * (g2_ + 1)]
        ranks = np.argsort(-c_, kind="stable")
        for o_ in range(GB // 2):
            ord_[GB * g2_ + 2 * o_] = GB * g2_ + ranks[o_]
            ord_[GB * g2_ + 2 * o_ + 1] = GB * g2_ + ranks[GB - 1 - o_]

    bf = ml_dtypes.bfloat16
    kcb = np.ascontiguousarray(
        k_cache.astype(bf).transpose(2, 0, 1, 3).reshape(HKV, NSLOTS, D)
    )
    vcb = np.ascontiguousarray(
        v_cache.astype(bf).transpose(2, 0, 1, 3).reshape(HKV, NSLOTS, D)
    )
    kcb[:, slot_mapping, :] = k.astype(bf).transpose(1, 0, 2)
    vcb[:, slot_mapping, :] = v.astype(bf).transpose(1, 0, 2)
    kcb = kcb.reshape(HKV, NUM_PAGES, PAGE * D)
    vcb = vcb.reshape(HKV, NUM_PAGES, PAGE * D)

    q = q[ord_]
    block_tables = block_tables[ord_]
    context_lens = context_lens[ord_]

    pos = np.arange(SMAX)
    invalid = pos[None, :] >= context_lens[:, None]  # [B, SMAX]

    cntp = np.minimum((context_lens + PAGE - 1) // PAGE, NP).astype(np.int32)
    i_ = np.arange(NP)
    pinvalid = i_[None, :] >= cntp[:, None]  # [B, NP]

    # V: pair tile idx: partition p -> page p of seq A (p<64) / p-64 of B
    vp = np.where(pinvalid, INVALID_IDX, block_tables[:, :NP]).astype(np.int32)  # [B, 64]
    vpidx = np.ascontiguousarray(
        vp.reshape(B // 2, 2 * NP).T
    )  # [128, 32]

    # K: per-pair idx: seq A 64 slots (invalid -> page 0 garbage), seq B
    # valid prefix then -1 (truncated). count = # non-negative.
    ka = np.where(pinvalid, 0, block_tables[:, :NP]).astype(np.int16)  # [B, 64]
    kb = np.where(pinvalid, -1, block_tables[:, :NP]).astype(np.int16)
    kidx = np.zeros((128, (B // 2) * 8), np.int16)
    kcnt = np.zeros((1, B // 2), np.int32)
    ii = np.arange(2 * NP)
    for o_ in range(B // 2):
        lin = np.concatenate([ka[2 * o_], kb[2 * o_ + 1]])
        for m_ in range(8):
            kidx[16 * m_ + ii % 16, 8 * o_ + ii // 16] = lin
        kcnt[0, o_] = int((lin >= 0).sum())

    # score column perm: col t*64 + i <-> position 32*i + t
    m = np.where(invalid, np.float32(-1e30), np.float32(0.0)).astype(np.float32)
    mperm = np.ascontiguousarray(
        m.reshape(B, NP, PAGE).transpose(0, 2, 1).reshape(B, SMAX)
    )
    maskadd = np.ascontiguousarray(
        np.repeat(mperm.reshape(GROUPS, GB, 1, SMAX), G, axis=2).reshape(
            GROUPS, GB * G, SMAX
        )
    )

    ident = np.eye(128, dtype=np.float32).astype(bf)

    bb = np.arange(B) % GB
    in_maps = []
    for h in range(NCORES):
        qh = q[:, G * h : G * h + G, :] * np.float32(SCALE)
        qTpad3 = np.zeros((B, D, 128), np.float32)
        for g in range(G):
            qTpad3[np.arange(B), :, 4 * bb + g] = qh[:, g, :]
        qTpad = np.ascontiguousarray(qTpad3.transpose(1, 0, 2).reshape(D, B * 128)).astype(bf)
        in_maps.append(
            {
                "kc": kcb[h],
                "vc": vcb[h],
                "qTpad": qTpad,
                "maskadd": maskadd,
                "vpidx": vpidx,
                "kidx": kidx,
                "kcnt": kcnt,
                "ident": ident,
            }
        )
    return in_maps, ord_


def assemble_out(raw_list):
    full = np.zeros((B, H, D), np.float32)
    for h, raw in enumerate(raw_list):
        r = np.asarray(raw).reshape(GROUPS, 4, 4, 8, D)  # [g2, q, g, u, d]
        full[:, 4 * h : 4 * h + 4, :] = (
            r.transpose(0, 3, 1, 2, 4).reshape(B, 4, D)
        )
    return full


def kernel(q, k, v, k_cache, v_cache, block_tables, context_lens, slot_mapping,
           trace=False, **trace_kwargs):
    in_maps, ord_ = make_in_maps(
        q, k, v, k_cache, v_cache, block_tables, context_lens, slot_mapping
    )
    nc = _get_nc()
    res = run_bass_kernel_spmd(
        nc, in_maps, core_ids=list(range(NCORES)), trace=trace, **trace_kwargs
    )
    perm = assemble_out([res.results[i]["out"] for i in range(NCORES)])
    full = np.empty_like(perm)
    full[ord_] = perm
    if trace:
        return full, res
    return full
